# revision 9
# baseline (speedup 1.0000x reference)
"""Cumulative link (ordinal) loss on 8 Trainium2 NeuronCores.

loss = mean_i [ -ln( sigmoid(hi_i - x_i) - sigmoid(lo_i - x_i) + eps ) ]
with per-label thresholds hi = [0,1,2,3,+inf][l], lo = [-inf,0,1,2,3][l].

Device formulation (lm = l - 3, H = lm - x, so G = l - x = H + 3):
    S1 = sigmoid(H + 3)      # = sigmoid(hi - x) when l <= 3   (ACT bias)
    S2 = sigmoid(H + 2)      # = sigmoid(lo - x) when l >= 1   (ACT bias)
    A  = max(lm, S1)         # l==4  ->  1,  else S1
    B  = min(lev, S2)        # l==0  ->  0,  else S2   (lev = lm + 3)
    P  = A - B
    sum ln(P_even * P_odd + eps) per chunk pair via ACT Ln accum_out;
    host negates / divides.  (p^2 >= 1.4e-5 so the 1e-8 eps only
    rescues exact-zero 16-bit sigmoid-saturation collisions.)

Perf structure:
  * All DMA on the single sync HWDGE ring (measured ~430 GB/s, 12.6
    MB/core in ~29us).  No SWDGE -- Q7 descriptor-gen contends with the
    DVE perf-mode SBUF ports and delays stream start.
  * Logits stay f32 in SBUF; DVE reads the high bf16 half of each f32
    word (stride-2 view = truncation, ~3e-4 rel on the mean, gate 2e-2).
  * 2048-column tiles end to end: halves the DVE/ACT instruction and
    semaphore count vs 1024 tiles (~8us off the DVE queue, the critical
    engine) and amortizes the per-op 58-cycle init.
  * Every DVE tensor_tensor except H is dense bf16 (2x perf mode);
    the old scalar_tensor_tensor clamp (measured 1x) is replaced by
    plain max/min against the shifted label tensors lm/lev, with the
    +3/+2 shifts folded into the sigmoid biases.
  * ACT program order: sigmoids tiles 0-2, ln pair 0 (table switch in
    DMA-gated slack), sigmoids tile 3, ln pair 1 -- the post-DMA tail
    is one sigmoid pair + one hidden table load + one ln.

Sharding: pure data parallel, 1/8 of batch per core, [128 x 8192].
"""

import numpy as np

B_TOTAL = 8388608
N_CORES = 8
P = 128
SHARD = B_TOTAL // N_CORES          # 1048576 per core
M = SHARD // P                      # 8192 free-dim columns per core
T = 2048                            # DMA chunk = compute tile width
NT = M // T                         # 4 tiles = 2 ln pairs
EPS = 1e-8

_NC = None


def _build_nc():
    import concourse.bacc as bacc
    import concourse.mybir as mybir
    from concourse import tile
    from concourse.tile_rust import add_dep_helper

    f32 = mybir.dt.float32
    bf16 = mybir.dt.bfloat16
    i32 = mybir.dt.int32
    Alu = mybir.AluOpType
    Act = mybir.ActivationFunctionType

    nc = bacc.Bacc("TRN2", target_bir_lowering=False, debug=False,
                   enable_asserts=False)

    x_dram = nc.dram_tensor("logits", (P, M), f32, kind="ExternalInput")
    # int32 pairs at the PJRT boundary (int64 inputs crash the axon run
    # path); low word of each pair is the label value.
    l_dram = nc.dram_tensor("labels", (P, 2 * M), i32, kind="ExternalInput")
    o_dram = nc.dram_tensor("out", (P, NT // 2), f32, kind="ExternalOutput")

    with tile.TileContext(nc) as tc:
        with tc.tile_pool(name="io", bufs=2) as iop, \
             tc.tile_pool(name="work", bufs=3) as wp, \
             tc.tile_pool(name="persist", bufs=1) as pp:
            bias3 = pp.tile([P, 1], f32, tag="bias3")
            nc.vector.memset(bias3[:], 3.0)
            bias2 = pp.tile([P, 1], f32, tag="bias2")
            nc.vector.memset(bias2[:], 2.0)
            bias_eps = pp.tile([P, 1], f32, tag="bias_eps")
            nc.vector.memset(bias_eps[:], EPS)
            ppf = pp.tile([P, (NT // 2) * T], bf16, tag="ppf")
            acc = pp.tile([P, NT // 2], f32, tag="acc")

            acts = []           # ACT instructions in intended program order
            p_tiles = [None, None]

            def emit_tile(t):
                l32 = iop.tile([P, T, 2], i32, tag="l32")
                xt = iop.tile([P, T, 2], bf16, tag="xt")
                nc.sync.dma_start(out=l32[:],
                                  in_=l_dram[:, 2 * T * t:2 * T * (t + 1)])
                nc.sync.dma_start(out=xt[:].bitcast(f32),
                                  in_=x_dram[:, T * t:T * (t + 1)])
                ls = l32[:, :, 0]       # int32 labels, stride 2
                xs = xt[:, :, 1]        # high bf16 of each f32
                lm = wp.tile([P, T], bf16, tag="lm")
                lev = wp.tile([P, T], bf16, tag="lev")
                h = wp.tile([P, T], bf16, tag="h")
                s1 = wp.tile([P, T], bf16, tag="s1")
                s2 = wp.tile([P, T], bf16, tag="s2")
                # lm = l - 3   (int32 strided -> dense bf16)
                nc.vector.tensor_scalar(out=lm[:], in0=ls, scalar1=-3.0,
                                        scalar2=None, op0=Alu.add)
                # lev = l      (dense single-src, 4x)
                nc.vector.tensor_scalar(out=lev[:], in0=lm[:], scalar1=3.0,
                                        scalar2=None, op0=Alu.add)
                # H = lm - x   (strided bf16 src: 1x, unavoidable)
                nc.vector.tensor_tensor(out=h[:], in0=lm[:], in1=xs,
                                        op=Alu.subtract)
                acts.append(nc.scalar.activation(s1[:], h[:], Act.Sigmoid,
                                                 bias=bias3[:]))
                acts.append(nc.scalar.activation(s2[:], h[:], Act.Sigmoid,
                                                 bias=bias2[:]))
                # A = max(lm, S1) -> s1's slot   (dense 2x)
                nc.vector.tensor_tensor(out=s1[:], in0=lm[:], in1=s1[:],
                                        op=Alu.max)
                # B = min(lev, S2) -> s2's slot  (dense 2x)
                nc.vector.tensor_tensor(out=s2[:], in0=lev[:], in1=s2[:],
                                        op=Alu.min)
                # P = A - B -> h's slot
                nc.vector.tensor_tensor(out=h[:], in0=s1[:], in1=s2[:],
                                        op=Alu.subtract)
                p_tiles[t % 2] = h
                if t % 2 == 1:
                    d = t // 2
                    nc.vector.tensor_tensor(
                        out=ppf[:, d * T:(d + 1) * T], in0=p_tiles[0][:],
                        in1=p_tiles[1][:], op=Alu.mult)

            def ln_pair(d):
                acts.append(nc.scalar.activation(
                    ppf[:, d * T:(d + 1) * T], ppf[:, d * T:(d + 1) * T],
                    Act.Ln, bias=bias_eps[:], accum_out=acc[:, d:d + 1]))

            emit_tile(0)
            emit_tile(1)
            emit_tile(2)
            ln_pair(0)
            emit_tile(3)
            ln_pair(1)

            # Freeze the ACT program order exactly as emitted, so ln pair 0
            # (and its table switches) runs inside the ACT engine's
            # DMA-gated slack instead of after the last sigmoid.
            for prev, nxt in zip(acts, acts[1:]):
                add_dep_helper(nxt.ins, prev.ins, sync=False,
                               reason="pin ACT order")

            nc.sync.dma_start(out=o_dram[:], in_=acc[:])

    nc.compile()
    return nc


def get_nc():
    global _NC
    if _NC is None:
        _NC = _build_nc()
    return _NC


def make_in_maps(logits, labels):
    x = np.ascontiguousarray(np.asarray(logits, dtype=np.float32)).reshape(B_TOTAL)
    lab = np.asarray(labels)
    if lab.dtype != np.int64:
        lab = lab.astype(np.int64)
    lab = np.ascontiguousarray(lab).reshape(B_TOTAL)
    in_maps = []
    for c in range(N_CORES):
        xs = x[c * SHARD:(c + 1) * SHARD].reshape(P, M)
        ls = lab[c * SHARD:(c + 1) * SHARD].view(np.int32).reshape(P, 2 * M)
        in_maps.append({"logits": xs, "labels": ls})
    return in_maps


def run(logits, labels, trace=False):
    """Returns (loss_scalar_f32, BassKernelResults)."""
    from concourse.bass_utils import run_bass_kernel_spmd

    nc = get_nc()
    in_maps = make_in_maps(logits, labels)
    res = run_bass_kernel_spmd(
        nc, in_maps, core_ids=list(range(N_CORES)), trace=trace
    )
    total = 0.0
    for r in res.results:
        total += r["out"].astype(np.float64).sum()
    loss = np.float32(-total / B_TOTAL)
    return np.asarray(loss), res


def kernel(logits, labels):
    out, _ = run(logits, labels, trace=False)
    return out
